# revision 34
# baseline (speedup 1.0000x reference)
"""Trainium2 Bass kernel for nn_MultiHeadAttention (fused QKV + RMS-norm +
RoPE + masked softmax attention + out-proj), tensor-parallel over heads
across 8 NeuronCores.

Contract: kernel(**inputs) takes FULL inputs, returns FULL output.
Self-contained: imports only numpy + the concourse framework.

v2: fp16 matmul operands (fp32 PSUM), single activation-table set
(Exp/Ln/Square/Copy), column-restricted diagonal blocks, head-interleaved
attention with out-proj filler, fp16 output DMA summed on host.
"""
import numpy as np

import concourse.bacc as bacc
import concourse.mybir as mybir
import concourse.tile as tile
from concourse import bass_utils

N_HEAD = 16
ROPE_BASE = 100000.0
RMS_EPS = 1e-5
L = 2048
D = 2048
HD = 128          # head dim
N_CORES = 8
HPD = N_HEAD // N_CORES   # heads per device = 2
QT = 512          # q tile (attention + l-tile width)
NQT = L // QT     # 4
NKB = L // 128    # 16 k-blocks
NEG = -1.0e9
SCALE = 1.0 / np.sqrt(HD)
ESHIFT = -2.0     # uniform exp shift; cancels in softmax, keeps pt in f16 range

F32 = mybir.dt.float32
F16 = mybir.dt.float16

_prog_cache: dict = {}


def _classify_mask(bm: np.ndarray):
    """Per (q-tile, k-block) classification of the boolean mask.

    plan[qt][kb] = ('full',) | ('skip',) | ('part', bias_idx, bstart, bw,
    zstart, zw): add biases[bias_idx][:, :bw] to score cols
    [bstart:bstart+bw], cols [zstart:zstart+zw] are fully masked. biases is
    [n, 128, 512] fp32 (col-sliced bias subtiles, zero-padded)."""
    plan = []
    uniq = {}
    biases = []
    for qt in range(NQT):
        row = []
        for kb in range(NKB):
            reg = bm[qt * QT:(qt + 1) * QT, kb * 128:(kb + 1) * 128]
            if reg.all():
                row.append(("full",))
                continue
            if not reg.any():
                row.append(("skip",))
                continue
            regT = reg.T  # [128 k, 512 q]
            col_all = regT.all(axis=0)       # fully open columns
            col_none = (~regT).all(axis=0)   # fully masked columns
            mixed = ~(col_all | col_none)
            mix_idx = np.nonzero(mixed)[0]
            none_idx = np.nonzero(col_none)[0]
            # contiguity of ranges (true for causal masks)
            ok = True
            bs = bw = zs = zw = 0
            if mix_idx.size:
                bs, be = int(mix_idx[0]), int(mix_idx[-1]) + 1
                bw = be - bs
                ok &= bool(mixed[bs:be].all())
            if none_idx.size:
                zs, ze = int(none_idx[0]), int(none_idx[-1]) + 1
                zw = ze - zs
                ok &= bool(col_none[zs:ze].all())
                ok &= not (bw and not (ze <= bs or be <= zs))
            if not ok:
                # fallback: full-width bias
                bs, bw, zs, zw = 0, QT, 0, 0
            if bw:
                bias = np.zeros((128, QT), np.float32)
                bias[:, 0:bw] = np.where(regT[:, bs:bs + bw], np.float32(0),
                                         np.float32(NEG))
                key = (bw, bias.tobytes())
                if key not in uniq:
                    uniq[key] = len(biases)
                    biases.append(bias)
                bi = uniq[key]
            else:
                bi = 0
            row.append(("part", bi, bs, bw, zs, zw))
        plan.append(tuple(row))
    if not biases:
        biases.append(np.zeros((128, QT), np.float32))
    return tuple(plan), np.stack(biases)


def _restrict(ent, first):
    """Start column for compute on this block (0 = full width).

    Only left-contiguous fully-masked column runs can be skipped, and the
    first block of an accumulation group must cover all columns."""
    if first or ent[0] != "part":
        return 0
    _, bi, bs, bw, zs, zw = ent
    if zw and zs == 0 and (bw == 0 or bs >= zw):
        return zw
    return 0


def _build_program(plan, n_bias):
    nc = bacc.Bacc("TRN2", target_bir_lowering=False, debug=False)

    # ---- DRAM I/O ----
    xT_d = nc.dram_tensor("xT", [D, L], F16, kind="ExternalInput")
    # host-packed first l-tile: row-block j holds d-chunks 2j,2j+1 side by
    # side (2KB DMA lines)
    xT0p_d = nc.dram_tensor("xT0p", [D // 2, 2 * QT], F16,
                            kind="ExternalInput")
    # host-packed: row-block j holds d-chunks 2j,2j+1 side by side (3KB lines)
    wqkvT_d = nc.dram_tensor("wqkvT", [D // 2, 2 * 6 * HD], F16,
                             kind="ExternalInput")
    woutT_d = nc.dram_tensor("woutT", [2 * HD, D], F16, kind="ExternalInput")
    cos_d = nc.dram_tensor("cos64", [64, L], F16, kind="ExternalInput")
    sin_d = nc.dram_tensor("sin64", [64, L], F16, kind="ExternalInput")
    ones128_d = nc.dram_tensor("ones128", [128, 1], F16, kind="ExternalInput")
    bias_d = nc.dram_tensor("biasT", [n_bias, 128, QT], F32, kind="ExternalInput")
    out_d = nc.dram_tensor("out", [L, D], F16, kind="ExternalOutput")

    Exp = mybir.ActivationFunctionType.Exp
    Ln = mybir.ActivationFunctionType.Ln
    Square = mybir.ActivationFunctionType.Square

    with nc.allow_low_precision(reason="fp16 operands, fp32 accumulation"), \
         tile.TileContext(nc) as tc:
        with (
            tc.tile_pool(name="const", bufs=1) as cpool,
            tc.tile_pool(name="xt", bufs=33) as xtpool,
            tc.tile_pool(name="act", bufs=1) as apool,
            tc.tile_pool(name="wrk", bufs=1) as wpool,
            tc.tile_pool(name="ps", bufs=1, space="PSUM") as ps,
        ):
            # ---- interleave first l-tile's x slices with weight slices so
            # the first projection chain starts as soon as pair 0 lands ----
            wt_all = cpool.tile([128, 16 * 6 * HD], F16, name="wt_all",
                                tag="wt_all")
            xts0 = []
            for j in range(8):
                xt = xtpool.tile([128, 2 * QT], F16, name="xt", tag="xt",
                                 bufs=8)
                nc.sync.dma_start(xt[:], xT0p_d.ap()[j * 128:(j + 1) * 128, :])
                xts0.append(xt)
                # q/k/v weight columns right behind their x slices (one
                # 3KB-line DMA covers two d-chunks)
                nc.sync.dma_start(
                    wt_all[:, 2 * j * 768:2 * (j + 1) * 768],
                    wqkvT_d.ap()[j * 128:(j + 1) * 128, :],
                )
            cos64 = cpool.tile([64, L], F16, name="cos64", tag="cos64")
            sin64 = cpool.tile([64, L], F16, name="sin64", tag="sin64")
            nc.sync.dma_start(cos64[:], cos_d.ap())
            nc.sync.dma_start(sin64[:], sin_d.ap())
            ones128 = cpool.tile([128, 1], F16, name="ones128", tag="ones128")
            nc.sync.dma_start(ones128[:], ones128_d.ap())
            epsc = cpool.tile([128, 1], F32, name="epsc", tag="epsc")
            nc.vector.memset(epsc[:], RMS_EPS)
            eshc = cpool.tile([128, 1], F32, name="eshc", tag="eshc")
            nc.vector.memset(eshc[:], ESHIFT)
            btiles = []
            for b in range(n_bias):
                bt = cpool.tile([128, QT], F32, name=f"bias{b}", tag=f"bias{b}")
                nc.sync.dma_start(bt[:], bias_d.ap()[b])
                btiles.append(bt)
            # x for l-tiles 1-3 as 3KB-line DMAs right behind the preamble;
            # W_out is deferred until l-tile 1 (first needed at l-tile 1)
            xts123 = []
            for i in range(16):
                xt = xtpool.tile([128, 3 * QT], F16, name="xt123",
                                 tag="xt123", bufs=16)
                nc.sync.dma_start(
                    xt[:], xT_d.ap()[i * 128:(i + 1) * 128, QT:4 * QT]
                )
                xts123.append(xt)
            wout_all = cpool.tile([128, 2 * D], F16, name="wout_all",
                                  tag="wout_all")

            # ---- persistent activations (fp16) ----
            ktr = [apool.tile([128, L], F16, name=f"ktr{h}", tag=f"ktr{h}")
                   for h in range(HPD)]
            aot = [apool.tile([128, L], F16, name=f"aot{h}", tag=f"aot{h}")
                   for h in range(HPD)]
            vnat = [apool.tile([128, 2 * HD], F16, name=f"vnat{lb}",
                               tag=f"vnat{lb}") for lb in range(16)]

            # ---------- out-projection unit (emitted as attention filler) ----
            def make_outproj_units(lt):
                units = []
                for j in range(4):
                    lb = 4 * lt + j
                    for jp in range(2):  # jt pairs (0,1) and (2,3)
                        def unit(lb=lb, jp=jp):
                            ob_sb = wpool.tile([128, 2 * QT], F16,
                                               name="ob_sb", tag="ob_sb",
                                               bufs=3)
                            for u in range(2):
                                jt = 2 * jp + u
                                fo = ps.tile([128, QT], F32,
                                             name=f"fo{lb}_{jt}",
                                             tag="pqkfo", bufs=2)
                                for h in range(2):
                                    nc.tensor.matmul(
                                        fo[:],
                                        aot[h][:, lb * 128:(lb + 1) * 128],
                                        wout_all[:, h * D + jt * QT:
                                                 h * D + (jt + 1) * QT],
                                        start=(h == 0), stop=(h == 1),
                                    )
                                dst = ob_sb[:, u * QT:(u + 1) * QT]
                                nc.vector.tensor_copy(out=dst, in_=fo[:])
                            nc.sync.dma_start(
                                out_d.ap()[lb * 128:(lb + 1) * 128,
                                           jp * 2 * QT:(jp + 1) * 2 * QT],
                                ob_sb[:],
                            )
                        units.append(unit)
                return units

            pending_units = []

            for lt in range(NQT):
                ls = lt * QT
                # ---------- QKV projection for this l-tile ----------
                if lt == 0:
                    xts = [(xts0[i // 2], (i % 2) * QT) for i in range(16)]
                else:
                    xts = [(t, (lt - 1) * QT) for t in xts123]
                if lt == 1:
                    for h in range(2):
                        nc.sync.dma_start(
                            wout_all[:, h * D:(h + 1) * D],
                            woutT_d.ap()[h * 128:(h + 1) * 128, :],
                        )

                # q/k chains: ob = 0,1 -> q heads 0,1 ; ob = 2,3 -> k heads 0,1
                # processed as q-pair then k-pair, each pair's RMS stats batch
                # (one Ln + one Exp) right behind it so q-rope starts while
                # the k chains are still on the PE
                pp = {}
                qtr_lt = []
                # q-pair first: q-rope hides under the k chains; the k-rope
                # tail has slack (this tile's k-blocks come last in attention)
                for t in (0, 1):
                    obs = (2 * t, 2 * t + 1)
                    for ob in obs:
                        tag, nb = (("pqkfo", 2) if ob < 2 else ("sT", 2)) \
                            if lt == 0 else (("pqkfo", 2))
                        pp[ob] = ps.tile([128, QT], F32,
                                         name=f"pqk{lt}_{ob}", tag=tag,
                                         bufs=nb)
                    if lt == 0:
                        # DMA-inflow-bound: i-outer over the pair
                        for i in range(16):
                            xt, xb = xts[i]
                            for ob in obs:
                                nc.tensor.matmul(
                                    pp[ob][:],
                                    wt_all[:, i * 768 + ob * 128:
                                           i * 768 + (ob + 1) * 128],
                                    xt[:, xb:xb + QT],
                                    start=(i == 0), stop=(i == 15),
                                )
                    else:
                        for ob in obs:
                            for i in range(16):
                                xt, xb = xts[i]
                                nc.tensor.matmul(
                                    pp[ob][:],
                                    wt_all[:, i * 768 + ob * 128:
                                           i * 768 + (ob + 1) * 128],
                                    xt[:, xb:xb + QT],
                                    start=(i == 0), stop=(i == 15),
                                )

                    # pair stats: sum-of-squares rows at partitions 0/64 (PE
                    # base-partition constraint), packed along one partition's
                    # free dim for a single Ln + Exp per pair
                    ssqp = ps.tile([65, QT], F32, name=f"ssqp_{lt}_{t}",
                                   tag="ssq", bufs=1)
                    ssq2 = wpool.tile([1, 2 * QT], F32, name="ssq2",
                                      tag="ssq2", bufs=2)
                    rawb = {}
                    for h, ob in enumerate(obs):
                        row = 64 * h
                        sq = wpool.tile([128, QT], F16, name="sq", tag="sq",
                                        bufs=2)
                        nc.scalar.activation(sq[:], pp[ob][:], Square)
                        rb16 = wpool.tile([128, QT], F16, name="rb16",
                                          tag="rb16", bufs=3)
                        nc.vector.tensor_copy(out=rb16[:], in_=pp[ob][:])
                        rawb[ob] = rb16
                        nc.tensor.matmul(
                            ssqp[row:row + 1, :], ones128[:], sq[:],
                            start=True, stop=True,
                        )
                        nc.vector.tensor_copy(
                            out=ssq2[0:1, h * QT:(h + 1) * QT],
                            in_=ssqp[row:row + 1, :])
                    # s4 = exp(-0.5*ln(ssq/HD + eps)) = 1/sqrt(mean+eps)
                    lg = wpool.tile([1, 2 * QT], F32, name="lg", tag="lg",
                                    bufs=2)
                    nc.scalar.activation(lg[:], ssq2[:], Ln,
                                         bias=epsc[0:1, :], scale=1.0 / HD)
                    s4 = wpool.tile([1, 2 * QT], F16, name="s4", tag="s4",
                                    bufs=2)
                    nc.scalar.activation(s4[:], lg[:], Exp, scale=-0.5)

                    # rms-scale + rope for the pair
                    for h, ob in enumerate(obs):
                        s2 = wpool.tile([128, QT], F16, name="s2", tag="s2",
                                        bufs=2)
                        nc.gpsimd.partition_broadcast(
                            s2[:], s4[0:1, h * QT:(h + 1) * QT])
                        scaled = wpool.tile([128, QT], F16, name="scaled",
                                            tag="scaled", bufs=1)
                        nc.vector.tensor_mul(out=scaled[:], in0=rawb[ob][:],
                                             in1=s2[:])
                        if t == 0:
                            dst = wpool.tile([128, QT], F16, name="qtr",
                                             tag="qtr", bufs=3)
                            od, ev = dst[0:64, :], dst[64:128, :]
                        else:
                            od = ktr[h][0:64, ls:ls + QT]
                            ev = ktr[h][64:128, ls:ls + QT]
                        ch = cos64[:, ls:ls + QT]
                        sh = sin64[:, ls:ls + QT]
                        shi = wpool.tile([64, QT], F16, name="shi", tag="shi",
                                         bufs=1)
                        nc.vector.tensor_copy(out=shi[:],
                                              in_=scaled[64:128, :])
                        t1 = wpool.tile([64, QT], F16, name="t1", tag="tsc1",
                                        bufs=1)
                        t2 = wpool.tile([64, QT], F16, name="t2", tag="tsc2",
                                        bufs=1)
                        nc.vector.tensor_mul(out=t1[:], in0=scaled[0:64, :],
                                             in1=ch)
                        nc.vector.tensor_mul(out=t2[:], in0=shi[:], in1=sh)
                        nc.vector.tensor_sub(out=od, in0=t1[:], in1=t2[:])
                        nc.vector.tensor_mul(out=t1[:], in0=scaled[0:64, :],
                                             in1=sh)
                        nc.vector.tensor_mul(out=t2[:], in0=shi[:], in1=ch)
                        nc.vector.tensor_add(out=ev, in0=t1[:], in1=t2[:])
                        if t == 0:
                            qtr_lt.append(dst)

                # v projection: natural layout [l, d] per 128-l block
                for j in range(4):
                    lb = 4 * lt + j
                    vp = ps.tile([128, QT], F32, name=f"vp{lb}", tag="pqkfo",
                                 bufs=2)
                    for i in range(16):
                        xt, xb = xts[i]
                        nc.tensor.matmul(
                            vp[:, 0:256],
                            xt[:, xb + j * 128:xb + (j + 1) * 128],
                            wt_all[:, i * 768 + 512: i * 768 + 768],
                            start=(i == 0), stop=(i == 15),
                        )
                    nc.vector.tensor_copy(out=vnat[lb][:], in_=vp[:, 0:256])

                # ---------- attention for q-tile qt = lt ----------
                qt = lt
                alive = [kb for kb in range(NKB) if plan[qt][kb][0] != "skip"]
                nblk = len(alive)
                nunits = len(pending_units)
                sums2 = ps.tile([65, QT], F32, name=f"sums2_{qt}", tag="sums2",
                                bufs=1)
                oT = [ps.tile([128, QT], F32, name=f"oT{qt}_{h}", tag="oT",
                              bufs=2) for h in range(2)]
                # filler prefix: guaranteed-ready PE work covering the
                # q-rope latency before the first score matmul
                emitted = 0
                while emitted < nunits // 3:
                    pending_units[emitted]()
                    emitted += 1
                for n, kb in enumerate(alive):
                    ent = plan[qt][kb]
                    zr = _restrict(ent, n == 0)
                    for h in range(2):
                        st = ps.tile([128, QT], F32, name=f"sT{qt}_{h}_{n}",
                                     tag="sT", bufs=2)
                        nc.tensor.matmul(
                            st[:, zr:],
                            ktr[h][:, kb * 128:(kb + 1) * 128],
                            qtr_lt[h][:, zr:],
                            start=True, stop=True,
                        )
                        if ent[0] == "part":
                            _, bi, bs, bw, zs, zw = ent
                            if bw:
                                nc.vector.tensor_add(
                                    out=st[:, bs:bs + bw],
                                    in0=st[:, bs:bs + bw],
                                    in1=btiles[bi][:, 0:bw])
                            if zw and zr == 0:
                                nc.vector.tensor_scalar_add(
                                    out=st[:, zs:zs + zw],
                                    in0=st[:, zs:zs + zw],
                                    scalar1=NEG)
                        pt = wpool.tile([128, QT], F16, name="pt", tag="pt",
                                        bufs=4)
                        nc.scalar.activation(pt[:, zr:], st[:, zr:], Exp,
                                             bias=eshc[:], scale=SCALE)
                        nc.tensor.matmul(
                            sums2[64 * h:64 * h + 1, zr:], ones128[:], pt[:, zr:],
                            start=(n == 0), stop=(n == nblk - 1),
                        )
                        nc.tensor.matmul(
                            oT[h][:, zr:],
                            vnat[kb][:, h * HD:(h + 1) * HD],
                            pt[:, zr:],
                            start=(n == 0), stop=(n == nblk - 1),
                        )
                    # interleave out-proj filler from the previous l-tile
                    want = nunits // 3 + (2 * nunits * (n + 1) // 3) // nblk
                    while emitted < want:
                        pending_units[emitted]()
                        emitted += 1
                while emitted < nunits:
                    pending_units[emitted]()
                    emitted += 1

                # normalize: aot[h][:, qt] = oT * (1/sums) bcast over partitions
                for h in range(2):
                    if h:
                        sumrow = wpool.tile([1, QT], F32, name="sumrow",
                                            tag="sumrow", bufs=2)
                        nc.vector.tensor_copy(out=sumrow[:],
                                              in_=sums2[64:65, :])
                        sum_in = sumrow
                    else:
                        sum_in = sums2[0:1, :]
                    rinv = wpool.tile([1, QT], F32, name="rinv", tag="rinv",
                                      bufs=2)
                    nc.vector.reciprocal_approx_fast(out=rinv[:],
                                                     in_=sum_in[0:1, :])
                    rb = wpool.tile([128, QT], F32, name="rb", tag="rb",
                                    bufs=2)
                    nc.gpsimd.partition_broadcast(rb[:], rinv[:])
                    nc.vector.tensor_mul(
                        out=aot[h][:, qt * QT:(qt + 1) * QT],
                        in0=oT[h][:], in1=rb[:],
                    )

                pending_units = make_outproj_units(lt)

            # last l-tile's out-projection
            for unit in pending_units:
                unit()

    nc.finalize()
    return nc


def _rope_perm(h):
    """Row order within one head's 128 q/k features: odd indices then even."""
    base = h * HD
    return np.concatenate([np.arange(1, HD, 2), np.arange(0, HD, 2)]) + base


def _host_prep(x, W_qkv, W_out):
    xT = np.ascontiguousarray(x[0].T.astype(np.float16))
    x16 = xT[:, 0:QT].reshape(16, 128, QT)
    xT0p = np.ascontiguousarray(
        np.concatenate([x16[0::2], x16[1::2]], axis=2).reshape(1024, 2 * QT)
    )
    inv_freq = 1.0 / (ROPE_BASE ** (np.arange(0, HD, 2, dtype=np.float64) / HD))
    ang = np.arange(L, dtype=np.float64)[:, None] * inv_freq[None, :]
    cos64 = np.ascontiguousarray(np.cos(ang).T.astype(np.float16))
    sin64 = np.ascontiguousarray(np.sin(ang).T.astype(np.float16))
    ones128 = np.ones((128, 1), np.float16)

    per_core = []
    for d in range(N_CORES):
        h0 = HPD * d
        rows_q = np.concatenate([_rope_perm(h0), _rope_perm(h0 + 1)])
        rows = np.concatenate(
            [rows_q, D + rows_q,
             2 * D + np.arange(h0 * HD, (h0 + 2) * HD)]
        )
        wl = W_qkv[rows, :]                                     # [768, 2048]
        wqkvT = wl.T.astype(np.float16)                         # [2048, 768]
        # pack d-chunk pairs side by side: [1024, 1536] with 3KB DMA lines
        w16 = wqkvT.reshape(16, 128, 768)
        wqkvT = np.ascontiguousarray(
            np.concatenate([w16[0::2], w16[1::2]], axis=2).reshape(1024, 1536)
        )
        woutT = np.ascontiguousarray(
            W_out[:, h0 * HD:(h0 + 2) * HD].T.astype(np.float16)
        )                                                       # [256, 2048]
        per_core.append((wqkvT, woutT))
    return xT, xT0p, cos64, sin64, ones128, per_core


def kernel(x, W_qkv, W_out, block_mask):
    x = np.asarray(x, dtype=np.float32)
    W_qkv = np.asarray(W_qkv, dtype=np.float32)
    W_out = np.asarray(W_out, dtype=np.float32)
    bm = np.asarray(block_mask).astype(bool)

    plan, biases = _classify_mask(bm)
    key = (plan, biases.shape[0])
    if key not in _prog_cache:
        _prog_cache[key] = _build_program(plan, biases.shape[0])
    nc = _prog_cache[key]

    xT, xT0p, cos64, sin64, ones128, per_core = _host_prep(x, W_qkv, W_out)
    in_maps = []
    for d in range(N_CORES):
        wqkvT, woutT = per_core[d]
        in_maps.append({
            "xT": xT, "xT0p": xT0p, "wqkvT": wqkvT, "woutT": woutT,
            "cos64": cos64, "sin64": sin64,
            "ones128": ones128, "biasT": biases,
        })
    res = bass_utils.run_bass_kernel_spmd(nc, in_maps, list(range(N_CORES)))
    acc = np.zeros((L, D), np.float64)
    for r in res.results:
        acc += r["out"].astype(np.float64)
    return acc.astype(np.float32)[None, :, :]


# revision 35
# speedup vs baseline: 1.1032x; 1.1032x over previous
"""Trainium2 Bass kernel for nn_MultiHeadAttention (fused QKV + RMS-norm +
RoPE + masked softmax attention + out-proj), tensor-parallel over heads
across 8 NeuronCores.

Contract: kernel(**inputs) takes FULL inputs, returns FULL output.
Self-contained: imports only numpy + the concourse framework.

v2: fp16 matmul operands (fp32 PSUM), single activation-table set
(Exp/Ln/Square/Copy), column-restricted diagonal blocks, head-interleaved
attention with out-proj filler, fp16 output DMA summed on host.
"""
import numpy as np

import concourse.bacc as bacc
import concourse.mybir as mybir
import concourse.tile as tile
from concourse import bass_utils

N_HEAD = 16
ROPE_BASE = 100000.0
RMS_EPS = 1e-5
L = 2048
D = 2048
HD = 128          # head dim
N_CORES = 8
HPD = N_HEAD // N_CORES   # heads per device = 2
QT = 512          # q tile (attention + l-tile width)
NQT = L // QT     # 4
NKB = L // 128    # 16 k-blocks
NEG = -1.0e9
SCALE = 1.0 / np.sqrt(HD)
ESHIFT = -2.0     # uniform exp shift; cancels in softmax, keeps pt in f16 range

F32 = mybir.dt.float32
F16 = mybir.dt.float16

_prog_cache: dict = {}


def _classify_mask(bm: np.ndarray):
    """Per (q-tile, k-block) classification of the boolean mask.

    plan[qt][kb] = ('full',) | ('skip',) | ('part', bias_idx, bstart, bw,
    zstart, zw): add biases[bias_idx][:, :bw] to score cols
    [bstart:bstart+bw], cols [zstart:zstart+zw] are fully masked. biases is
    [n, 128, 512] fp32 (col-sliced bias subtiles, zero-padded)."""
    plan = []
    uniq = {}
    biases = []
    for qt in range(NQT):
        row = []
        for kb in range(NKB):
            reg = bm[qt * QT:(qt + 1) * QT, kb * 128:(kb + 1) * 128]
            if reg.all():
                row.append(("full",))
                continue
            if not reg.any():
                row.append(("skip",))
                continue
            regT = reg.T  # [128 k, 512 q]
            col_all = regT.all(axis=0)       # fully open columns
            col_none = (~regT).all(axis=0)   # fully masked columns
            mixed = ~(col_all | col_none)
            mix_idx = np.nonzero(mixed)[0]
            none_idx = np.nonzero(col_none)[0]
            # contiguity of ranges (true for causal masks)
            ok = True
            bs = bw = zs = zw = 0
            if mix_idx.size:
                bs, be = int(mix_idx[0]), int(mix_idx[-1]) + 1
                bw = be - bs
                ok &= bool(mixed[bs:be].all())
            if none_idx.size:
                zs, ze = int(none_idx[0]), int(none_idx[-1]) + 1
                zw = ze - zs
                ok &= bool(col_none[zs:ze].all())
                ok &= not (bw and not (ze <= bs or be <= zs))
            if not ok:
                # fallback: full-width bias
                bs, bw, zs, zw = 0, QT, 0, 0
            if bw:
                bias = np.zeros((128, QT), np.float32)
                bias[:, 0:bw] = np.where(regT[:, bs:bs + bw], np.float32(0),
                                         np.float32(NEG))
                key = (bw, bias.tobytes())
                if key not in uniq:
                    uniq[key] = len(biases)
                    biases.append(bias)
                bi = uniq[key]
            else:
                bi = 0
            row.append(("part", bi, bs, bw, zs, zw))
        plan.append(tuple(row))
    if not biases:
        biases.append(np.zeros((128, QT), np.float32))
    return tuple(plan), np.stack(biases)


def _restrict(ent, first):
    """Start column for compute on this block (0 = full width).

    Only left-contiguous fully-masked column runs can be skipped, and the
    first block of an accumulation group must cover all columns."""
    if first or ent[0] != "part":
        return 0
    _, bi, bs, bw, zs, zw = ent
    if zw and zs == 0 and (bw == 0 or bs >= zw):
        return zw
    return 0


def _build_program(plan, n_bias):
    nc = bacc.Bacc("TRN2", target_bir_lowering=False, debug=False)

    # ---- DRAM I/O ----
    xT_d = nc.dram_tensor("xT", [D, L], F16, kind="ExternalInput")
    # host-packed first l-tile: row-block j holds d-chunks 2j,2j+1 side by
    # side (2KB DMA lines)
    xT0p_d = nc.dram_tensor("xT0p", [D // 2, 2 * QT], F16,
                            kind="ExternalInput")
    # host-packed: row-block j holds d-chunks 2j,2j+1 side by side (3KB lines)
    wqkvT_d = nc.dram_tensor("wqkvT", [D // 2, 2 * 6 * HD], F16,
                             kind="ExternalInput")
    woutT_d = nc.dram_tensor("woutT", [2 * HD, D], F16, kind="ExternalInput")
    cos_d = nc.dram_tensor("cos64", [64, L], F16, kind="ExternalInput")
    sin_d = nc.dram_tensor("sin64", [64, L], F16, kind="ExternalInput")
    ones128_d = nc.dram_tensor("ones128", [128, 1], F16, kind="ExternalInput")
    bias_d = nc.dram_tensor("biasT", [n_bias, 128, QT], F32, kind="ExternalInput")
    out_d = nc.dram_tensor("out", [L, D], F16, kind="ExternalOutput")

    Exp = mybir.ActivationFunctionType.Exp
    Ln = mybir.ActivationFunctionType.Ln
    Square = mybir.ActivationFunctionType.Square

    with nc.allow_low_precision(reason="fp16 operands, fp32 accumulation"), \
         tile.TileContext(nc) as tc:
        with (
            tc.tile_pool(name="const", bufs=1) as cpool,
            tc.tile_pool(name="xt", bufs=33) as xtpool,
            tc.tile_pool(name="act", bufs=1) as apool,
            tc.tile_pool(name="wrk", bufs=1) as wpool,
            tc.tile_pool(name="ps", bufs=1, space="PSUM") as ps,
        ):
            # ---- interleave first l-tile's x slices with weight slices so
            # the first projection chain starts as soon as pair 0 lands ----
            wt_all = cpool.tile([128, 16 * 6 * HD], F16, name="wt_all",
                                tag="wt_all")
            xts0 = []
            for j in range(8):
                xt = xtpool.tile([128, 2 * QT], F16, name="xt", tag="xt",
                                 bufs=8)
                nc.sync.dma_start(xt[:], xT0p_d.ap()[j * 128:(j + 1) * 128, :])
                xts0.append(xt)
                # q/k/v weight columns right behind their x slices (one
                # 3KB-line DMA covers two d-chunks)
                nc.sync.dma_start(
                    wt_all[:, 2 * j * 768:2 * (j + 1) * 768],
                    wqkvT_d.ap()[j * 128:(j + 1) * 128, :],
                )
            cos64 = cpool.tile([64, L], F16, name="cos64", tag="cos64")
            sin64 = cpool.tile([64, L], F16, name="sin64", tag="sin64")
            nc.sync.dma_start(cos64[:], cos_d.ap())
            nc.sync.dma_start(sin64[:], sin_d.ap())
            ones128 = cpool.tile([128, 1], F16, name="ones128", tag="ones128")
            nc.sync.dma_start(ones128[:], ones128_d.ap())
            epsc = cpool.tile([128, 1], F32, name="epsc", tag="epsc")
            nc.vector.memset(epsc[:], RMS_EPS)
            eshc = cpool.tile([128, 1], F32, name="eshc", tag="eshc")
            nc.vector.memset(eshc[:], ESHIFT)
            btiles = []
            for b in range(n_bias):
                bt = cpool.tile([128, QT], F32, name=f"bias{b}", tag=f"bias{b}")
                nc.sync.dma_start(bt[:], bias_d.ap()[b])
                btiles.append(bt)
            # x for l-tiles 1-3 as 3KB-line DMAs right behind the preamble;
            # W_out is deferred until l-tile 1 (first needed at l-tile 1)
            xts123 = []
            for i in range(16):
                xt = xtpool.tile([128, 3 * QT], F16, name="xt123",
                                 tag="xt123", bufs=16)
                # scalar-engine ring: parallel to the sync-ring preamble
                nc.scalar.dma_start(
                    xt[:], xT_d.ap()[i * 128:(i + 1) * 128, QT:4 * QT]
                )
                xts123.append(xt)
            wout_all = cpool.tile([128, 2 * D], F16, name="wout_all",
                                  tag="wout_all")
            for h in range(2):
                nc.sync.dma_start(
                    wout_all[:, h * D:(h + 1) * D],
                    woutT_d.ap()[h * 128:(h + 1) * 128, :],
                )

            # ---- persistent activations (fp16) ----
            ktr = [apool.tile([128, L], F16, name=f"ktr{h}", tag=f"ktr{h}")
                   for h in range(HPD)]
            aot = [apool.tile([128, L], F16, name=f"aot{h}", tag=f"aot{h}")
                   for h in range(HPD)]
            vnat = [apool.tile([128, 2 * HD], F16, name=f"vnat{lb}",
                               tag=f"vnat{lb}") for lb in range(16)]

            # ---------- out-projection unit (emitted as attention filler) ----
            def make_outproj_units(lt):
                units = []
                for j in range(4):
                    lb = 4 * lt + j
                    for jp in range(2):  # jt pairs (0,1) and (2,3)
                        def unit(lb=lb, jp=jp):
                            ob_sb = wpool.tile([128, 2 * QT], F16,
                                               name="ob_sb", tag="ob_sb",
                                               bufs=6)
                            for u in range(2):
                                jt = 2 * jp + u
                                fo = ps.tile([128, QT], F32,
                                             name=f"fo{lb}_{jt}",
                                             tag="pqkfo", bufs=2)
                                for h in range(2):
                                    nc.tensor.matmul(
                                        fo[:],
                                        aot[h][:, lb * 128:(lb + 1) * 128],
                                        wout_all[:, h * D + jt * QT:
                                                 h * D + (jt + 1) * QT],
                                        start=(h == 0), stop=(h == 1),
                                    )
                                dst = ob_sb[:, u * QT:(u + 1) * QT]
                                nc.vector.tensor_copy(out=dst, in_=fo[:])
                            nc.sync.dma_start(
                                out_d.ap()[lb * 128:(lb + 1) * 128,
                                           jp * 2 * QT:(jp + 1) * 2 * QT],
                                ob_sb[:],
                            )
                        units.append(unit)
                return units

            pending_units = []

            for lt in range(NQT):
                ls = lt * QT
                # ---------- QKV projection for this l-tile ----------
                if lt == 0:
                    xts = [(xts0[i // 2], (i % 2) * QT) for i in range(16)]
                else:
                    xts = [(t, (lt - 1) * QT) for t in xts123]

                # q/k chains: ob = 0,1 -> q heads 0,1 ; ob = 2,3 -> k heads 0,1
                # processed as q-pair then k-pair, each pair's RMS stats batch
                # (one Ln + one Exp) right behind it so q-rope starts while
                # the k chains are still on the PE
                pp = {}
                qtr_lt = []
                # q-pair first: q-rope hides under the k chains; the k-rope
                # tail has slack (this tile's k-blocks come last in attention)
                for t in (0, 1):
                    obs = (2 * t, 2 * t + 1)
                    for ob in obs:
                        tag, nb = (("pqkfo", 2) if ob < 2 else ("sT", 2)) \
                            if lt == 0 else (("pqkfo", 2))
                        pp[ob] = ps.tile([128, QT], F32,
                                         name=f"pqk{lt}_{ob}", tag=tag,
                                         bufs=nb)
                    if lt == 0:
                        # DMA-inflow-bound: i-outer over the pair
                        for i in range(16):
                            xt, xb = xts[i]
                            for ob in obs:
                                nc.tensor.matmul(
                                    pp[ob][:],
                                    wt_all[:, i * 768 + ob * 128:
                                           i * 768 + (ob + 1) * 128],
                                    xt[:, xb:xb + QT],
                                    start=(i == 0), stop=(i == 15),
                                )
                    else:
                        for ob in obs:
                            for i in range(16):
                                xt, xb = xts[i]
                                nc.tensor.matmul(
                                    pp[ob][:],
                                    wt_all[:, i * 768 + ob * 128:
                                           i * 768 + (ob + 1) * 128],
                                    xt[:, xb:xb + QT],
                                    start=(i == 0), stop=(i == 15),
                                )

                    # pair stats: sum-of-squares rows at partitions 0/64 (PE
                    # base-partition constraint), packed along one partition's
                    # free dim for a single Ln + Exp per pair
                    ssqp = ps.tile([65, QT], F32, name=f"ssqp_{lt}_{t}",
                                   tag="ssq", bufs=1)
                    ssq2 = wpool.tile([1, 2 * QT], F32, name="ssq2",
                                      tag="ssq2", bufs=2)
                    rawb = {}
                    for h, ob in enumerate(obs):
                        row = 64 * h
                        sq = wpool.tile([128, QT], F16, name="sq", tag="sq",
                                        bufs=2)
                        nc.scalar.activation(sq[:], pp[ob][:], Square)
                        rb16 = wpool.tile([128, QT], F16, name="rb16",
                                          tag="rb16", bufs=3)
                        nc.vector.tensor_copy(out=rb16[:], in_=pp[ob][:])
                        rawb[ob] = rb16
                        nc.tensor.matmul(
                            ssqp[row:row + 1, :], ones128[:], sq[:],
                            start=True, stop=True,
                        )
                        nc.vector.tensor_copy(
                            out=ssq2[0:1, h * QT:(h + 1) * QT],
                            in_=ssqp[row:row + 1, :])
                    # s4 = exp(-0.5*ln(ssq/HD + eps)) = 1/sqrt(mean+eps)
                    lg = wpool.tile([1, 2 * QT], F32, name="lg", tag="lg",
                                    bufs=2)
                    nc.scalar.activation(lg[:], ssq2[:], Ln,
                                         bias=epsc[0:1, :], scale=1.0 / HD)
                    s4 = wpool.tile([1, 2 * QT], F16, name="s4", tag="s4",
                                    bufs=2)
                    nc.scalar.activation(s4[:], lg[:], Exp, scale=-0.5)

                    # rms-scale + rope for the pair
                    for h, ob in enumerate(obs):
                        s2 = wpool.tile([128, QT], F16, name="s2", tag="s2",
                                        bufs=2)
                        nc.gpsimd.partition_broadcast(
                            s2[:], s4[0:1, h * QT:(h + 1) * QT])
                        scaled = wpool.tile([128, QT], F16, name="scaled",
                                            tag="scaled", bufs=1)
                        nc.vector.tensor_mul(out=scaled[:], in0=rawb[ob][:],
                                             in1=s2[:])
                        if t == 0:
                            dst = wpool.tile([128, QT], F16, name="qtr",
                                             tag="qtr", bufs=3)
                            od, ev = dst[0:64, :], dst[64:128, :]
                        else:
                            od = ktr[h][0:64, ls:ls + QT]
                            ev = ktr[h][64:128, ls:ls + QT]
                        ch = cos64[:, ls:ls + QT]
                        sh = sin64[:, ls:ls + QT]
                        shi = wpool.tile([64, QT], F16, name="shi", tag="shi",
                                         bufs=1)
                        nc.vector.tensor_copy(out=shi[:],
                                              in_=scaled[64:128, :])
                        t1 = wpool.tile([64, QT], F16, name="t1", tag="tsc1",
                                        bufs=1)
                        t2 = wpool.tile([64, QT], F16, name="t2", tag="tsc2",
                                        bufs=1)
                        nc.vector.tensor_mul(out=t1[:], in0=scaled[0:64, :],
                                             in1=ch)
                        nc.vector.tensor_mul(out=t2[:], in0=shi[:], in1=sh)
                        nc.vector.tensor_sub(out=od, in0=t1[:], in1=t2[:])
                        nc.vector.tensor_mul(out=t1[:], in0=scaled[0:64, :],
                                             in1=sh)
                        nc.vector.tensor_mul(out=t2[:], in0=shi[:], in1=ch)
                        nc.vector.tensor_add(out=ev, in0=t1[:], in1=t2[:])
                        if t == 0:
                            qtr_lt.append(dst)

                # v projection: natural layout [l, d] per 128-l block
                for j in range(4):
                    lb = 4 * lt + j
                    vp = ps.tile([128, QT], F32, name=f"vp{lb}", tag="pqkfo",
                                 bufs=2)
                    for i in range(16):
                        xt, xb = xts[i]
                        nc.tensor.matmul(
                            vp[:, 0:256],
                            xt[:, xb + j * 128:xb + (j + 1) * 128],
                            wt_all[:, i * 768 + 512: i * 768 + 768],
                            start=(i == 0), stop=(i == 15),
                        )
                    nc.vector.tensor_copy(out=vnat[lb][:], in_=vp[:, 0:256])

                # ---------- attention for q-tile qt = lt ----------
                qt = lt
                alive = [kb for kb in range(NKB) if plan[qt][kb][0] != "skip"]
                nblk = len(alive)
                nunits = len(pending_units)
                sums2 = ps.tile([65, QT], F32, name=f"sums2_{qt}", tag="sums2",
                                bufs=1)
                oT = [ps.tile([128, QT], F32, name=f"oT{qt}_{h}", tag="oT",
                              bufs=2) for h in range(2)]
                # filler prefix: guaranteed-ready PE work covering the
                # q-rope latency before the first score matmul
                emitted = 0
                while emitted < nunits // 4:
                    pending_units[emitted]()
                    emitted += 1
                for n, kb in enumerate(alive):
                    ent = plan[qt][kb]
                    zr = _restrict(ent, n == 0)
                    for h in range(2):
                        st = ps.tile([128, QT], F32, name=f"sT{qt}_{h}_{n}",
                                     tag="sT", bufs=2)
                        nc.tensor.matmul(
                            st[:, zr:],
                            ktr[h][:, kb * 128:(kb + 1) * 128],
                            qtr_lt[h][:, zr:],
                            start=True, stop=True,
                        )
                        if ent[0] == "part":
                            _, bi, bs, bw, zs, zw = ent
                            if bw:
                                nc.vector.tensor_add(
                                    out=st[:, bs:bs + bw],
                                    in0=st[:, bs:bs + bw],
                                    in1=btiles[bi][:, 0:bw])
                            if zw and zr == 0:
                                nc.vector.tensor_scalar_add(
                                    out=st[:, zs:zs + zw],
                                    in0=st[:, zs:zs + zw],
                                    scalar1=NEG)
                        pt = wpool.tile([128, QT], F16, name="pt", tag="pt",
                                        bufs=4)
                        nc.scalar.activation(pt[:, zr:], st[:, zr:], Exp,
                                             bias=eshc[:], scale=SCALE)
                        nc.tensor.matmul(
                            sums2[64 * h:64 * h + 1, zr:], ones128[:], pt[:, zr:],
                            start=(n == 0), stop=(n == nblk - 1),
                        )
                        nc.tensor.matmul(
                            oT[h][:, zr:],
                            vnat[kb][:, h * HD:(h + 1) * HD],
                            pt[:, zr:],
                            start=(n == 0), stop=(n == nblk - 1),
                        )
                    # interleave out-proj filler from the previous l-tile
                    want = nunits // 4 + (3 * nunits * (n + 1) // 4) // nblk
                    while emitted < want:
                        pending_units[emitted]()
                        emitted += 1
                while emitted < nunits:
                    pending_units[emitted]()
                    emitted += 1

                # normalize: aot[h][:, qt] = oT * (1/sums) bcast over partitions
                for h in range(2):
                    if h:
                        sumrow = wpool.tile([1, QT], F32, name="sumrow",
                                            tag="sumrow", bufs=2)
                        nc.vector.tensor_copy(out=sumrow[:],
                                              in_=sums2[64:65, :])
                        sum_in = sumrow
                    else:
                        sum_in = sums2[0:1, :]
                    rinv = wpool.tile([1, QT], F32, name="rinv", tag="rinv",
                                      bufs=2)
                    nc.vector.reciprocal_approx_fast(out=rinv[:],
                                                     in_=sum_in[0:1, :])
                    rb = wpool.tile([128, QT], F32, name="rb", tag="rb",
                                    bufs=2)
                    nc.gpsimd.partition_broadcast(rb[:], rinv[:])
                    nc.vector.tensor_mul(
                        out=aot[h][:, qt * QT:(qt + 1) * QT],
                        in0=oT[h][:], in1=rb[:],
                    )

                pending_units = make_outproj_units(lt)

            # last l-tile's out-projection
            for unit in pending_units:
                unit()

    nc.finalize()
    return nc


def _rope_perm(h):
    """Row order within one head's 128 q/k features: odd indices then even."""
    base = h * HD
    return np.concatenate([np.arange(1, HD, 2), np.arange(0, HD, 2)]) + base


def _host_prep(x, W_qkv, W_out):
    xT = np.ascontiguousarray(x[0].T.astype(np.float16))
    x16 = xT[:, 0:QT].reshape(16, 128, QT)
    xT0p = np.ascontiguousarray(
        np.concatenate([x16[0::2], x16[1::2]], axis=2).reshape(1024, 2 * QT)
    )
    inv_freq = 1.0 / (ROPE_BASE ** (np.arange(0, HD, 2, dtype=np.float64) / HD))
    ang = np.arange(L, dtype=np.float64)[:, None] * inv_freq[None, :]
    cos64 = np.ascontiguousarray(np.cos(ang).T.astype(np.float16))
    sin64 = np.ascontiguousarray(np.sin(ang).T.astype(np.float16))
    ones128 = np.ones((128, 1), np.float16)

    per_core = []
    for d in range(N_CORES):
        h0 = HPD * d
        rows_q = np.concatenate([_rope_perm(h0), _rope_perm(h0 + 1)])
        rows = np.concatenate(
            [rows_q, D + rows_q,
             2 * D + np.arange(h0 * HD, (h0 + 2) * HD)]
        )
        wl = W_qkv[rows, :]                                     # [768, 2048]
        wqkvT = wl.T.astype(np.float16)                         # [2048, 768]
        # pack d-chunk pairs side by side: [1024, 1536] with 3KB DMA lines
        w16 = wqkvT.reshape(16, 128, 768)
        wqkvT = np.ascontiguousarray(
            np.concatenate([w16[0::2], w16[1::2]], axis=2).reshape(1024, 1536)
        )
        woutT = np.ascontiguousarray(
            W_out[:, h0 * HD:(h0 + 2) * HD].T.astype(np.float16)
        )                                                       # [256, 2048]
        per_core.append((wqkvT, woutT))
    return xT, xT0p, cos64, sin64, ones128, per_core


def kernel(x, W_qkv, W_out, block_mask):
    x = np.asarray(x, dtype=np.float32)
    W_qkv = np.asarray(W_qkv, dtype=np.float32)
    W_out = np.asarray(W_out, dtype=np.float32)
    bm = np.asarray(block_mask).astype(bool)

    plan, biases = _classify_mask(bm)
    key = (plan, biases.shape[0])
    if key not in _prog_cache:
        _prog_cache[key] = _build_program(plan, biases.shape[0])
    nc = _prog_cache[key]

    xT, xT0p, cos64, sin64, ones128, per_core = _host_prep(x, W_qkv, W_out)
    in_maps = []
    for d in range(N_CORES):
        wqkvT, woutT = per_core[d]
        in_maps.append({
            "xT": xT, "xT0p": xT0p, "wqkvT": wqkvT, "woutT": woutT,
            "cos64": cos64, "sin64": sin64,
            "ones128": ones128, "biasT": biases,
        })
    res = bass_utils.run_bass_kernel_spmd(nc, in_maps, list(range(N_CORES)))
    acc = np.zeros((L, D), np.float64)
    for r in res.results:
        acc += r["out"].astype(np.float64)
    return acc.astype(np.float32)[None, :, :]


# revision 36
# speedup vs baseline: 1.1707x; 1.0611x over previous
"""Trainium2 Bass kernel for nn_MultiHeadAttention (fused QKV + RMS-norm +
RoPE + masked softmax attention + out-proj), tensor-parallel over heads
across 8 NeuronCores.

Contract: kernel(**inputs) takes FULL inputs, returns FULL output.
Self-contained: imports only numpy + the concourse framework.

v2: fp16 matmul operands (fp32 PSUM), single activation-table set
(Exp/Ln/Square/Copy), column-restricted diagonal blocks, head-interleaved
attention with out-proj filler, fp16 output DMA summed on host.
"""
import numpy as np

import concourse.bacc as bacc
import concourse.mybir as mybir
import concourse.tile as tile
from concourse import bass_utils

N_HEAD = 16
ROPE_BASE = 100000.0
RMS_EPS = 1e-5
L = 2048
D = 2048
HD = 128          # head dim
N_CORES = 8
HPD = N_HEAD // N_CORES   # heads per device = 2
QT = 512          # q tile (attention + l-tile width)
NQT = L // QT     # 4
NKB = L // 128    # 16 k-blocks
NEG = -1.0e9
SCALE = 1.0 / np.sqrt(HD)
ESHIFT = -2.0     # uniform exp shift; cancels in softmax, keeps pt in f16 range

F32 = mybir.dt.float32
F16 = mybir.dt.float16

_prog_cache: dict = {}


def _classify_mask(bm: np.ndarray):
    """Per (q-tile, k-block) classification of the boolean mask.

    plan[qt][kb] = ('full',) | ('skip',) | ('part', bias_idx, bstart, bw,
    zstart, zw): add biases[bias_idx][:, :bw] to score cols
    [bstart:bstart+bw], cols [zstart:zstart+zw] are fully masked. biases is
    [n, 128, 512] fp32 (col-sliced bias subtiles, zero-padded)."""
    plan = []
    uniq = {}
    biases = []
    for qt in range(NQT):
        row = []
        for kb in range(NKB):
            reg = bm[qt * QT:(qt + 1) * QT, kb * 128:(kb + 1) * 128]
            if reg.all():
                row.append(("full",))
                continue
            if not reg.any():
                row.append(("skip",))
                continue
            regT = reg.T  # [128 k, 512 q]
            col_all = regT.all(axis=0)       # fully open columns
            col_none = (~regT).all(axis=0)   # fully masked columns
            mixed = ~(col_all | col_none)
            mix_idx = np.nonzero(mixed)[0]
            none_idx = np.nonzero(col_none)[0]
            # contiguity of ranges (true for causal masks)
            ok = True
            bs = bw = zs = zw = 0
            if mix_idx.size:
                bs, be = int(mix_idx[0]), int(mix_idx[-1]) + 1
                bw = be - bs
                ok &= bool(mixed[bs:be].all())
            if none_idx.size:
                zs, ze = int(none_idx[0]), int(none_idx[-1]) + 1
                zw = ze - zs
                ok &= bool(col_none[zs:ze].all())
                ok &= not (bw and not (ze <= bs or be <= zs))
            if not ok:
                # fallback: full-width bias
                bs, bw, zs, zw = 0, QT, 0, 0
            if bw:
                bias = np.zeros((128, QT), np.float32)
                bias[:, 0:bw] = np.where(regT[:, bs:bs + bw], np.float32(0),
                                         np.float32(NEG))
                key = (bw, bias.tobytes())
                if key not in uniq:
                    uniq[key] = len(biases)
                    biases.append(bias)
                bi = uniq[key]
            else:
                bi = 0
            row.append(("part", bi, bs, bw, zs, zw))
        plan.append(tuple(row))
    if not biases:
        biases.append(np.zeros((128, QT), np.float32))
    return tuple(plan), np.stack(biases)


def _restrict(ent, first):
    """Start column for compute on this block (0 = full width).

    Only left-contiguous fully-masked column runs can be skipped, and the
    first block of an accumulation group must cover all columns."""
    if first or ent[0] != "part":
        return 0
    _, bi, bs, bw, zs, zw = ent
    if zw and zs == 0 and (bw == 0 or bs >= zw):
        return zw
    return 0


def _build_program(plan, n_bias):
    nc = bacc.Bacc("TRN2", target_bir_lowering=False, debug=False)

    # ---- DRAM I/O ----
    xT_d = nc.dram_tensor("xT", [D, L], F16, kind="ExternalInput")
    # host-packed: row-block j holds d-chunks 2j,2j+1 side by side (3KB lines)
    wqkvT_d = nc.dram_tensor("wqkvT", [D // 2, 2 * 6 * HD], F16,
                             kind="ExternalInput")
    woutT_d = nc.dram_tensor("woutT", [2 * HD, D], F16, kind="ExternalInput")
    cos_d = nc.dram_tensor("cos64", [64, L], F16, kind="ExternalInput")
    sin_d = nc.dram_tensor("sin64", [64, L], F16, kind="ExternalInput")
    ones128_d = nc.dram_tensor("ones128", [128, 1], F16, kind="ExternalInput")
    bias_d = nc.dram_tensor("biasT", [n_bias, 128, QT], F32, kind="ExternalInput")
    out_d = nc.dram_tensor("out", [L, D], F16, kind="ExternalOutput")

    Exp = mybir.ActivationFunctionType.Exp
    Ln = mybir.ActivationFunctionType.Ln
    Square = mybir.ActivationFunctionType.Square

    with nc.allow_low_precision(reason="fp16 operands, fp32 accumulation"), \
         tile.TileContext(nc) as tc:
        with (
            tc.tile_pool(name="const", bufs=1) as cpool,
            tc.tile_pool(name="xt", bufs=33) as xtpool,
            tc.tile_pool(name="act", bufs=1) as apool,
            tc.tile_pool(name="wrk", bufs=1) as wpool,
            tc.tile_pool(name="ps", bufs=1, space="PSUM") as ps,
        ):
            # ---- interleave first l-tile's x slices with weight slices so
            # the first projection chain starts as soon as pair 0 lands ----
            wt_all = cpool.tile([128, 16 * 6 * HD], F16, name="wt_all",
                                tag="wt_all")
            xts0 = []
            for i in range(16):
                xt = xtpool.tile([128, QT], F16, name="xt", tag="xt",
                                 bufs=17)
                nc.sync.dma_start(xt[:], xT_d.ap()[i * 128:(i + 1) * 128, 0:QT])
                xts0.append(xt)
                # q/k/v weight columns right behind their x slice (one
                # 3KB-line DMA covers two d-chunks)
                if i % 2 == 0:
                    j = i // 2
                    nc.sync.dma_start(
                        wt_all[:, i * 768:(i + 2) * 768],
                        wqkvT_d.ap()[j * 128:(j + 1) * 128, :],
                    )
            cos64 = cpool.tile([64, L], F16, name="cos64", tag="cos64")
            sin64 = cpool.tile([64, L], F16, name="sin64", tag="sin64")
            nc.sync.dma_start(cos64[:], cos_d.ap())
            nc.sync.dma_start(sin64[:], sin_d.ap())
            ones128 = cpool.tile([128, 1], F16, name="ones128", tag="ones128")
            nc.sync.dma_start(ones128[:], ones128_d.ap())
            epsc = cpool.tile([128, 1], F32, name="epsc", tag="epsc")
            nc.vector.memset(epsc[:], RMS_EPS)
            eshc = cpool.tile([128, 1], F32, name="eshc", tag="eshc")
            nc.vector.memset(eshc[:], ESHIFT)
            btiles = []
            for b in range(n_bias):
                bt = cpool.tile([128, QT], F32, name=f"bias{b}", tag=f"bias{b}")
                nc.sync.dma_start(bt[:], bias_d.ap()[b])
                btiles.append(bt)
            wout_all = cpool.tile([128, 2 * D], F16, name="wout_all",
                                  tag="wout_all")
            for h in range(2):
                nc.sync.dma_start(
                    wout_all[:, h * D:(h + 1) * D],
                    woutT_d.ap()[h * 128:(h + 1) * 128, :],
                )

            # ---- persistent activations (fp16) ----
            ktr = [apool.tile([128, L], F16, name=f"ktr{h}", tag=f"ktr{h}")
                   for h in range(HPD)]
            aot = [apool.tile([128, L], F16, name=f"aot{h}", tag=f"aot{h}")
                   for h in range(HPD)]
            vnat = [apool.tile([128, 2 * HD], F16, name=f"vnat{lb}",
                               tag=f"vnat{lb}") for lb in range(16)]

            # ---------- out-projection unit (emitted as attention filler) ----
            def make_outproj_units(lt):
                units = []
                for j in range(4):
                    lb = 4 * lt + j
                    for jp in range(2):  # jt pairs (0,1) and (2,3)
                        def unit(lb=lb, jp=jp):
                            ob_sb = wpool.tile([128, 2 * QT], F16,
                                               name="ob_sb", tag="ob_sb",
                                               bufs=6)
                            for u in range(2):
                                jt = 2 * jp + u
                                fo = ps.tile([128, QT], F32,
                                             name=f"fo{lb}_{jt}",
                                             tag="pqkfo", bufs=2)
                                for h in range(2):
                                    nc.tensor.matmul(
                                        fo[:],
                                        aot[h][:, lb * 128:(lb + 1) * 128],
                                        wout_all[:, h * D + jt * QT:
                                                 h * D + (jt + 1) * QT],
                                        start=(h == 0), stop=(h == 1),
                                    )
                                dst = ob_sb[:, u * QT:(u + 1) * QT]
                                nc.vector.tensor_copy(out=dst, in_=fo[:])
                            nc.sync.dma_start(
                                out_d.ap()[lb * 128:(lb + 1) * 128,
                                           jp * 2 * QT:(jp + 1) * 2 * QT],
                                ob_sb[:],
                            )
                        units.append(unit)
                return units

            pending_units = []
            xts2 = None

            for lt in range(NQT):
                ls = lt * QT
                # ---------- QKV projection for this l-tile ----------
                if lt == 0:
                    xts = [(t, 0) for t in xts0]
                elif lt == 1:
                    xts = []
                    for i in range(16):
                        xt = xtpool.tile([128, QT], F16, name="xt", tag="xt",
                                         bufs=17)
                        nc.sync.dma_start(
                            xt[:], xT_d.ap()[i * 128:(i + 1) * 128, ls:ls + QT]
                        )
                        xts.append((xt, 0))
                elif lt == 2:
                    xts2 = []
                    for i in range(16):
                        xt = xtpool.tile([128, 2 * QT], F16, name="xt2",
                                         tag="xt2", bufs=16)
                        nc.sync.dma_start(
                            xt[:],
                            xT_d.ap()[i * 128:(i + 1) * 128, ls:ls + 2 * QT]
                        )
                        xts2.append(xt)
                    xts = [(t, 0) for t in xts2]
                else:
                    xts = [(t, QT) for t in xts2]

                # q/k chains: ob = 0,1 -> q heads 0,1 ; ob = 2,3 -> k heads 0,1
                # processed as q-pair then k-pair, each pair's RMS stats batch
                # (one Ln + one Exp) right behind it so q-rope starts while
                # the k chains are still on the PE
                pp = {}
                qtr_lt = []
                # q-pair first: q-rope hides under the k chains; the k-rope
                # tail has slack (this tile's k-blocks come last in attention)
                for t in (0, 1):
                    obs = (2 * t, 2 * t + 1)
                    for ob in obs:
                        tag, nb = (("pqkfo", 2) if ob < 2 else ("sT", 2)) \
                            if lt == 0 else (("pqkfo", 2))
                        pp[ob] = ps.tile([128, QT], F32,
                                         name=f"pqk{lt}_{ob}", tag=tag,
                                         bufs=nb)
                    if lt == 0:
                        # DMA-inflow-bound: i-outer over the pair
                        for i in range(16):
                            xt, xb = xts[i]
                            for ob in obs:
                                nc.tensor.matmul(
                                    pp[ob][:],
                                    wt_all[:, i * 768 + ob * 128:
                                           i * 768 + (ob + 1) * 128],
                                    xt[:, xb:xb + QT],
                                    start=(i == 0), stop=(i == 15),
                                )
                    else:
                        for ob in obs:
                            for i in range(16):
                                xt, xb = xts[i]
                                nc.tensor.matmul(
                                    pp[ob][:],
                                    wt_all[:, i * 768 + ob * 128:
                                           i * 768 + (ob + 1) * 128],
                                    xt[:, xb:xb + QT],
                                    start=(i == 0), stop=(i == 15),
                                )

                    # pair stats: sum-of-squares rows at partitions 0/64 (PE
                    # base-partition constraint), packed along one partition's
                    # free dim for a single Ln + Exp per pair
                    ssqp = ps.tile([65, QT], F32, name=f"ssqp_{lt}_{t}",
                                   tag="ssq", bufs=1)
                    ssq2 = wpool.tile([1, 2 * QT], F32, name="ssq2",
                                      tag="ssq2", bufs=2)
                    rawb = {}
                    for h, ob in enumerate(obs):
                        row = 64 * h
                        sq = wpool.tile([128, QT], F16, name="sq", tag="sq",
                                        bufs=2)
                        nc.scalar.activation(sq[:], pp[ob][:], Square)
                        rb16 = wpool.tile([128, QT], F16, name="rb16",
                                          tag="rb16", bufs=3)
                        nc.vector.tensor_copy(out=rb16[:], in_=pp[ob][:])
                        rawb[ob] = rb16
                        nc.tensor.matmul(
                            ssqp[row:row + 1, :], ones128[:], sq[:],
                            start=True, stop=True,
                        )
                        nc.vector.tensor_copy(
                            out=ssq2[0:1, h * QT:(h + 1) * QT],
                            in_=ssqp[row:row + 1, :])
                    # s4 = exp(-0.5*ln(ssq/HD + eps)) = 1/sqrt(mean+eps)
                    lg = wpool.tile([1, 2 * QT], F32, name="lg", tag="lg",
                                    bufs=2)
                    nc.scalar.activation(lg[:], ssq2[:], Ln,
                                         bias=epsc[0:1, :], scale=1.0 / HD)
                    s4 = wpool.tile([1, 2 * QT], F16, name="s4", tag="s4",
                                    bufs=2)
                    nc.scalar.activation(s4[:], lg[:], Exp, scale=-0.5)

                    # rms-scale + rope for the pair
                    for h, ob in enumerate(obs):
                        s2 = wpool.tile([128, QT], F16, name="s2", tag="s2",
                                        bufs=2)
                        nc.gpsimd.partition_broadcast(
                            s2[:], s4[0:1, h * QT:(h + 1) * QT])
                        scaled = wpool.tile([128, QT], F16, name="scaled",
                                            tag="scaled", bufs=1)
                        nc.vector.tensor_mul(out=scaled[:], in0=rawb[ob][:],
                                             in1=s2[:])
                        if t == 0:
                            dst = wpool.tile([128, QT], F16, name="qtr",
                                             tag="qtr", bufs=3)
                            od, ev = dst[0:64, :], dst[64:128, :]
                        else:
                            od = ktr[h][0:64, ls:ls + QT]
                            ev = ktr[h][64:128, ls:ls + QT]
                        ch = cos64[:, ls:ls + QT]
                        sh = sin64[:, ls:ls + QT]
                        shi = wpool.tile([64, QT], F16, name="shi", tag="shi",
                                         bufs=1)
                        nc.vector.tensor_copy(out=shi[:],
                                              in_=scaled[64:128, :])
                        t1 = wpool.tile([64, QT], F16, name="t1", tag="tsc1",
                                        bufs=1)
                        t2 = wpool.tile([64, QT], F16, name="t2", tag="tsc2",
                                        bufs=1)
                        nc.vector.tensor_mul(out=t1[:], in0=scaled[0:64, :],
                                             in1=ch)
                        nc.vector.tensor_mul(out=t2[:], in0=shi[:], in1=sh)
                        nc.vector.tensor_sub(out=od, in0=t1[:], in1=t2[:])
                        nc.vector.tensor_mul(out=t1[:], in0=scaled[0:64, :],
                                             in1=sh)
                        nc.vector.tensor_mul(out=t2[:], in0=shi[:], in1=ch)
                        nc.vector.tensor_add(out=ev, in0=t1[:], in1=t2[:])
                        if t == 0:
                            qtr_lt.append(dst)

                # v projection: natural layout [l, d] per 128-l block
                for j in range(4):
                    lb = 4 * lt + j
                    vp = ps.tile([128, QT], F32, name=f"vp{lb}", tag="pqkfo",
                                 bufs=2)
                    for i in range(16):
                        xt, xb = xts[i]
                        nc.tensor.matmul(
                            vp[:, 0:256],
                            xt[:, xb + j * 128:xb + (j + 1) * 128],
                            wt_all[:, i * 768 + 512: i * 768 + 768],
                            start=(i == 0), stop=(i == 15),
                        )
                    nc.vector.tensor_copy(out=vnat[lb][:], in_=vp[:, 0:256])

                # ---------- attention for q-tile qt = lt ----------
                qt = lt
                alive = [kb for kb in range(NKB) if plan[qt][kb][0] != "skip"]
                nblk = len(alive)
                nunits = len(pending_units)
                sums2 = ps.tile([65, QT], F32, name=f"sums2_{qt}", tag="sums2",
                                bufs=1)
                oT = [ps.tile([128, QT], F32, name=f"oT{qt}_{h}", tag="oT",
                              bufs=2) for h in range(2)]
                emitted = 0
                for n, kb in enumerate(alive):
                    ent = plan[qt][kb]
                    zr = _restrict(ent, n == 0)
                    for h in range(2):
                        st = ps.tile([128, QT], F32, name=f"sT{qt}_{h}_{n}",
                                     tag="sT", bufs=2)
                        nc.tensor.matmul(
                            st[:, zr:],
                            ktr[h][:, kb * 128:(kb + 1) * 128],
                            qtr_lt[h][:, zr:],
                            start=True, stop=True,
                        )
                        if ent[0] == "part":
                            _, bi, bs, bw, zs, zw = ent
                            if bw:
                                nc.vector.tensor_add(
                                    out=st[:, bs:bs + bw],
                                    in0=st[:, bs:bs + bw],
                                    in1=btiles[bi][:, 0:bw])
                            if zw and zr == 0:
                                nc.vector.tensor_scalar_add(
                                    out=st[:, zs:zs + zw],
                                    in0=st[:, zs:zs + zw],
                                    scalar1=NEG)
                        pt = wpool.tile([128, QT], F16, name="pt", tag="pt",
                                        bufs=4)
                        nc.scalar.activation(pt[:, zr:], st[:, zr:], Exp,
                                             bias=eshc[:], scale=SCALE)
                        nc.tensor.matmul(
                            sums2[64 * h:64 * h + 1, zr:], ones128[:], pt[:, zr:],
                            start=(n == 0), stop=(n == nblk - 1),
                        )
                        nc.tensor.matmul(
                            oT[h][:, zr:],
                            vnat[kb][:, h * HD:(h + 1) * HD],
                            pt[:, zr:],
                            start=(n == 0), stop=(n == nblk - 1),
                        )
                    # interleave out-proj filler from the previous l-tile
                    want = (nunits * (n + 1)) // nblk
                    while emitted < want:
                        pending_units[emitted]()
                        emitted += 1
                while emitted < nunits:
                    pending_units[emitted]()
                    emitted += 1

                # normalize: aot[h][:, qt] = oT * (1/sums) bcast over partitions
                for h in range(2):
                    if h:
                        sumrow = wpool.tile([1, QT], F32, name="sumrow",
                                            tag="sumrow", bufs=2)
                        nc.vector.tensor_copy(out=sumrow[:],
                                              in_=sums2[64:65, :])
                        sum_in = sumrow
                    else:
                        sum_in = sums2[0:1, :]
                    rinv = wpool.tile([1, QT], F32, name="rinv", tag="rinv",
                                      bufs=2)
                    nc.vector.reciprocal_approx_fast(out=rinv[:],
                                                     in_=sum_in[0:1, :])
                    rb = wpool.tile([128, QT], F32, name="rb", tag="rb",
                                    bufs=2)
                    nc.gpsimd.partition_broadcast(rb[:], rinv[:])
                    nc.vector.tensor_mul(
                        out=aot[h][:, qt * QT:(qt + 1) * QT],
                        in0=oT[h][:], in1=rb[:],
                    )

                pending_units = make_outproj_units(lt)

            # last l-tile's out-projection
            for unit in pending_units:
                unit()

    nc.finalize()
    return nc


def _rope_perm(h):
    """Row order within one head's 128 q/k features: odd indices then even."""
    base = h * HD
    return np.concatenate([np.arange(1, HD, 2), np.arange(0, HD, 2)]) + base


def _host_prep(x, W_qkv, W_out):
    xT = np.ascontiguousarray(x[0].T.astype(np.float16))
    inv_freq = 1.0 / (ROPE_BASE ** (np.arange(0, HD, 2, dtype=np.float64) / HD))
    ang = np.arange(L, dtype=np.float64)[:, None] * inv_freq[None, :]
    cos64 = np.ascontiguousarray(np.cos(ang).T.astype(np.float16))
    sin64 = np.ascontiguousarray(np.sin(ang).T.astype(np.float16))
    ones128 = np.ones((128, 1), np.float16)

    per_core = []
    for d in range(N_CORES):
        h0 = HPD * d
        rows_q = np.concatenate([_rope_perm(h0), _rope_perm(h0 + 1)])
        rows = np.concatenate(
            [rows_q, D + rows_q,
             2 * D + np.arange(h0 * HD, (h0 + 2) * HD)]
        )
        wl = W_qkv[rows, :]                                     # [768, 2048]
        wqkvT = wl.T.astype(np.float16)                         # [2048, 768]
        # pack d-chunk pairs side by side: [1024, 1536] with 3KB DMA lines
        w16 = wqkvT.reshape(16, 128, 768)
        wqkvT = np.ascontiguousarray(
            np.concatenate([w16[0::2], w16[1::2]], axis=2).reshape(1024, 1536)
        )
        woutT = np.ascontiguousarray(
            W_out[:, h0 * HD:(h0 + 2) * HD].T.astype(np.float16)
        )                                                       # [256, 2048]
        per_core.append((wqkvT, woutT))
    return xT, cos64, sin64, ones128, per_core


def kernel(x, W_qkv, W_out, block_mask):
    x = np.asarray(x, dtype=np.float32)
    W_qkv = np.asarray(W_qkv, dtype=np.float32)
    W_out = np.asarray(W_out, dtype=np.float32)
    bm = np.asarray(block_mask).astype(bool)

    plan, biases = _classify_mask(bm)
    key = (plan, biases.shape[0])
    if key not in _prog_cache:
        _prog_cache[key] = _build_program(plan, biases.shape[0])
    nc = _prog_cache[key]

    xT, cos64, sin64, ones128, per_core = _host_prep(x, W_qkv, W_out)
    in_maps = []
    for d in range(N_CORES):
        wqkvT, woutT = per_core[d]
        in_maps.append({
            "xT": xT, "wqkvT": wqkvT, "woutT": woutT,
            "cos64": cos64, "sin64": sin64,
            "ones128": ones128, "biasT": biases,
        })
    res = bass_utils.run_bass_kernel_spmd(nc, in_maps, list(range(N_CORES)))
    acc = np.zeros((L, D), np.float64)
    for r in res.results:
        acc += r["out"].astype(np.float64)
    return acc.astype(np.float32)[None, :, :]
